# revision 1
# baseline (speedup 1.0000x reference)
"""CTC loss kernel for Trainium2 (8 NeuronCores, data-parallel over batch).

Math: per-sample CTC forward DP in the *linear* probability domain with
periodic joint renormalization (per-partition Z tracked in log space), so
the inner loop is pure multiply/add — no per-step logsumexp.

Layout: samples on partitions (128/chunk, 4 chunks/core packed along the
free dim). Per t-block of 32 steps: strided DMA load of preds in
(t-part, [n, c]) layout -> GPSIMD ap_gather picks the 65 extended-label
classes per sample (indices are time-invariant, shared across the
16-partition groups as the ISA requires) -> PE transposes flip to
(n-part, ...) -> ACT drains PSUM with exp() fused (prescale ln(32) as
activation bias) -> DVE runs 4 fused ops per DP step.
"""

import os

import numpy as np

import concourse.bass as bass
import concourse.bacc as bacc
import concourse.mybir as mybir
from concourse import masks, tile
from concourse.bass_utils import run_bass_kernel_spmd

# Problem shape (hardcoded per contract).
N, T, C, S = 4096, 128, 128, 32
S2 = 2 * S + 1          # 65 extended states
NCORES = 8
NPC = N // NCORES       # 512 samples per core
CH = 4                  # sample chunks per core
CHN = NPC // CH         # 128 samples per chunk
TBL = 32                # timesteps per t-block
TB = T // TBL           # 4 t-blocks
SW = S2 + 2             # state width incl 2 zero-pad cols at front
LN_SCALE = float(np.log(32.0))   # per-step prob prescale, keeps alpha ~O(1)
NORM_EVERY = 8

F32 = mybir.dt.float32
DT_E = mybir.dt.float32          # dtype of gathered prob tensor (En)
DT_ST = mybir.dt.float32         # dtype of DP state

_CACHE = {}
LAST_RESULTS = None


def _build_nc(expiry_steps):
    """Build the single-core Bass program (SPMD across 8 cores)."""
    nc = bacc.Bacc("TRN2", target_bir_lowering=False, debug=False)

    preds = nc.declare_dram_parameter("preds", [NPC, T, C], F32, isOutput=False)
    gidx = nc.declare_dram_parameter("gidx", [128, CHN * S2 // 16], mybir.dt.int16,
                                     isOutput=False)
    msk = nc.declare_dram_parameter("msk", [128, CH, S2], DT_ST, isOutput=False)
    imask = nc.declare_dram_parameter("imask", [128, CH, S2], DT_ST, isOutput=False)
    fmask = nc.declare_dram_parameter("fmask", [128, CH, S2], DT_ST, isOutput=False)
    nE = len(expiry_steps)
    expm = nc.declare_dram_parameter("expm", [128, CH, nE], F32, isOutput=False)
    out = nc.declare_dram_parameter("out", [128, CH], F32, isOutput=True)

    with tile.TileContext(nc) as tc:
        with (
            tc.tile_pool(name="const", bufs=1) as constp,
            tc.tile_pool(name="state", bufs=1) as statep,
            tc.tile_pool(name="data", bufs=1) as datap,
            tc.tile_pool(name="gath", bufs=1) as gathp,
            tc.tile_pool(name="en", bufs=2) as enp,
            tc.tile_pool(name="ps", bufs=4, space="PSUM") as psp,
        ):
            # ---- constants into SBUF ----
            t_gidx = constp.tile([128, CHN * S2 // 16], mybir.dt.int16, tag="gidx")
            nc.sync.dma_start(t_gidx[:], gidx[:, :])
            t_msk = constp.tile([128, CH, S2], DT_ST, tag="msk")
            nc.sync.dma_start(t_msk[:], msk[:, :, :])
            t_im = constp.tile([128, CH, S2], DT_ST, tag="im")
            nc.sync.dma_start(t_im[:], imask[:, :, :])
            t_fm = constp.tile([128, CH, S2], DT_ST, tag="fm")
            nc.sync.dma_start(t_fm[:], fmask[:, :, :])
            t_expm = constp.tile([128, CH, nE], F32, tag="expm")
            nc.sync.dma_start(t_expm[:], expm[:, :, :])
            ident = constp.tile([128, 128], F32, tag="ident")
            masks.make_identity(nc, ident[:])
            t_bias = constp.tile([128, 1], F32, tag="bias")
            nc.vector.memset(t_bias[:], LN_SCALE)

            # ---- persistent state ----
            stA = statep.tile([128, CH, SW], DT_ST, tag="stA")
            stB = statep.tile([128, CH, SW], DT_ST, tag="stB")
            tV = statep.tile([128, CH, S2], DT_ST, tag="tV")
            tC = statep.tile([128, 1], F32, tag="tC")       # sum of applied ln(Z)
            tZ = statep.tile([128, 1], F32, tag="tZ")
            tZinv = statep.tile([128, 1], F32, tag="tZinv")
            tLnZ = statep.tile([128, 1], F32, tag="tLnZ")
            tOut = statep.tile([128, CH], F32, tag="tOut")
            tR = statep.tile([128, CH], F32, tag="tR")
            tLr = statep.tile([128, CH], F32, tag="tLr")
            tCtb = statep.tile([128, CH], F32, tag="tCtb")

            nc.vector.memset(stA[:], 0.0)
            nc.vector.memset(stB[:], 0.0)
            nc.vector.memset(tC[:], 0.0)
            nc.vector.memset(tOut[:], 0.0)

            cur, nxt = stA, stB
            pending_norm = False  # Z captured last step; apply 1/Z this step

            def capture(alpha, ei):
                """Read out finished samples after the DP step for expiry ei."""
                nc.vector.tensor_mul(tV[:], alpha[:, :, 2:SW], t_fm[:])
                nc.vector.tensor_reduce(tR[:], tV[:],
                                        axis=mybir.AxisListType.X,
                                        op=mybir.AluOpType.add)
                nc.scalar.activation(tLr[:], tR[:],
                                     mybir.ActivationFunctionType.Ln)
                nc.vector.scalar_tensor_tensor(
                    out=tCtb[:], in0=tLr[:], scalar=tC[:, 0:1],
                    in1=t_expm[:, :, ei],
                    op0=mybir.AluOpType.add, op1=mybir.AluOpType.mult)
                nc.vector.tensor_add(tOut[:], tOut[:], tCtb[:])

            for tb in range(TB):
                # load this t-block: (t-part 128=[c(4) x tl(32)], [n(128), c(128)])
                data = datap.tile([128, CHN * C], F32, tag="data")
                dv = data[:].rearrange("p (n c) -> p n c", n=CHN)
                for c in range(CH):
                    src = preds[c * CHN:(c + 1) * CHN,
                                tb * TBL:(tb + 1) * TBL, :]
                    nc.sync.dma_start(out=dv[c * TBL:(c + 1) * TBL, :, :],
                                      in_=src.rearrange("n t c -> t n c"))

                # gather the 65 extended-label log-probs per sample
                G = gathp.tile([128, CHN * S2], F32, tag="G")
                nc.gpsimd.ap_gather(
                    out_ap=G[:], in_ap=data[:], idxs_ap=t_gidx[:],
                    channels=128, num_elems=CHN * C, d=1, num_idxs=CHN * S2)

                # transpose to (n-part) + fused exp into En
                en = enp.tile([128, CH, TBL, S2], DT_E, tag="en")
                gv = G[:].rearrange("p (n s) -> p s n", s=S2)
                for sb in range(0, S2, 4):
                    sc = min(4, S2 - sb)
                    ps = psp.tile([128, 4, 128], F32, tag="ps")
                    for i in range(sc):
                        nc.tensor.transpose(ps[:, i, :], gv[:, sb + i, :],
                                            ident[:])
                    dst = en[:, :, :, sb:sb + sc].rearrange("p c t s -> p s c t")
                    src = ps[:, 0:sc, :].rearrange("p s (c t) -> p s c t", c=CH)
                    nc.scalar.activation(dst, src,
                                         mybir.ActivationFunctionType.Exp,
                                         bias=t_bias[:, 0:1])

                # DP steps for this t-block
                for tl in range(TBL):
                    t = tb * TBL + tl
                    P = en[:, :, tl, :]  # (128, CH, S2)
                    if t == 0:
                        nc.vector.tensor_mul(cur[:, :, 2:SW], P, t_im[:])
                    else:
                        if pending_norm:
                            nc.vector.reciprocal(tZinv[:], tZ[:])
                            nc.scalar.activation(
                                tLnZ[:], tZ[:], mybir.ActivationFunctionType.Ln)
                            nc.vector.tensor_add(tC[:], tC[:], tLnZ[:])
                        want_z = (t % NORM_EVERY == NORM_EVERY - 1) and t < T - 1
                        nc.vector.tensor_add(nxt[:, :, 2:SW],
                                             cur[:, :, 2:SW], cur[:, :, 1:SW - 1])
                        nc.vector.tensor_mul(tV[:], cur[:, :, 0:SW - 2], t_msk[:])
                        nc.vector.tensor_add(nxt[:, :, 2:SW],
                                             nxt[:, :, 2:SW], tV[:])
                        nc.vector.scalar_tensor_tensor(
                            out=nxt[:, :, 2:SW], in0=nxt[:, :, 2:SW],
                            scalar=(tZinv[:, 0:1] if pending_norm else 1.0),
                            in1=P,
                            op0=mybir.AluOpType.mult, op1=mybir.AluOpType.mult,
                            accum_out=(tZ[:, 0:1] if want_z else None))
                        pending_norm = want_z
                        cur, nxt = nxt, cur
                    if t in expiry_steps:
                        capture(cur, expiry_steps.index(t))

            nc.sync.dma_start(out=out[:, :], in_=tOut[:])

    nc.compile()
    return nc


def _host_prep(preds, targets, pred_lengths, target_lengths):
    """Build per-core input maps + global expiry info. All O(N*S) host work."""
    preds = np.ascontiguousarray(np.asarray(preds, dtype=np.float32))
    targets = np.asarray(targets).astype(np.int64)
    pl = np.asarray(pred_lengths).astype(np.int64)
    tl = np.asarray(target_lengths).astype(np.int64)

    ext = np.zeros((N, S2), dtype=np.int64)
    ext[:, 1::2] = targets                      # blanks at even positions
    ext_m2 = np.full((N, S2), -1, dtype=np.int64)
    ext_m2[:, 2:] = ext[:, :-2]
    allow = (ext != 0) & (ext != ext_m2)        # skip-transition mask

    exp_step = np.clip(pl - 1, 0, T - 1)
    expiry_steps = sorted(set(int(e) for e in exp_step))

    in_maps = []
    for k in range(NCORES):
        sl = slice(k * NPC, (k + 1) * NPC)
        p_k = preds[sl]
        ext_k = ext[sl]
        # gather indices: per chunk c, list L[n*65+s] = n*128 + ext, wrapped
        # into 16-partition groups (2 groups per chunk share the list).
        gidx = np.zeros((128, CHN * S2 // 16), dtype=np.int16)
        for c in range(CH):
            e = ext_k[c * CHN:(c + 1) * CHN]                    # (128, 65)
            L = (np.arange(CHN)[:, None] * C + e).reshape(-1)   # (8320,)
            wrapped = L.reshape(-1, 16).T                       # (16, 520)
            gidx[2 * c * 16:(2 * c + 1) * 16] = wrapped
            gidx[(2 * c + 1) * 16:(2 * c + 2) * 16] = wrapped

        def pack(a):  # (NPC, S2) -> (128, CH, S2) partition-major
            return np.ascontiguousarray(
                a.reshape(CH, CHN, S2).transpose(1, 0, 2))

        mskv = pack(allow[sl].astype(np.float32))
        imv = np.zeros((NPC, S2), dtype=np.float32)
        imv[:, 0:2] = 1.0
        imv = pack(imv)
        fmv = np.zeros((NPC, S2), dtype=np.float32)
        tk = tl[sl]
        fmv[np.arange(NPC), 2 * tk] = 1.0
        fmv[np.arange(NPC), 2 * tk - 1] = 1.0
        fmv = pack(fmv)

        es_k = exp_step[sl]
        expmv = np.zeros((NPC, len(expiry_steps)), dtype=np.float32)
        for i, e in enumerate(expiry_steps):
            expmv[:, i] = (es_k == e)
        expmv = np.ascontiguousarray(
            expmv.reshape(CH, CHN, -1).transpose(1, 0, 2))

        in_maps.append({
            "preds": p_k,
            "gidx": gidx,
            "msk": mskv.astype(np.float32),
            "imask": imv.astype(np.float32),
            "fmask": fmv.astype(np.float32),
            "expm": expmv,
        })
    return in_maps, expiry_steps, pl, tl


def kernel(preds, targets, pred_lengths, target_lengths):
    in_maps, expiry_steps, pl, tl = _host_prep(
        preds, targets, pred_lengths, target_lengths)

    key = tuple(expiry_steps)
    if key not in _CACHE:
        _CACHE[key] = _build_nc(expiry_steps)
    nc = _CACHE[key]

    trace = bool(os.environ.get("CTC_TRACE"))
    res = run_bass_kernel_spmd(nc, in_maps, list(range(NCORES)), trace=trace)
    global LAST_RESULTS
    LAST_RESULTS = res
    outs = [res.results[i]["out"] for i in range(NCORES)]  # each (128, CH)

    logp = np.concatenate(
        [o.T.reshape(-1) for o in outs])       # (N,) chunk-major per core
    n_mult = np.clip(pl, 1, T).astype(np.float64)
    nll = -(logp.astype(np.float64) - n_mult * LN_SCALE)
    nll = np.where(nll >= 0.5e30, 0.0, nll)    # zero_infinity
    loss = np.float32(np.mean(nll / tl.astype(np.float64)))
    return np.asarray(loss, dtype=np.float32)

